# revision 21
# baseline (speedup 1.0000x reference)
"""BoundaryFluxAttention TRN2 kernel (v3: ACT-paced, deadline-driven filler).

Distribution (8 cores): data-parallel over batch (B=2) x tensor-parallel over
heads (16 heads -> 4 groups of 4). Core c handles batch c//4, head group c%4.
Each core computes a partial output y_c = softmax-attention(its 4 heads) @ W_out
rows for those heads; the host sums the 4 partials per batch and adds b_out.

v3 design (trace-driven rework of the v2 flat pipeline; baseline 204.6us):
  - The ACT exp chain is the de-facto critical path: 128 exps of [128,1024]
    at ~1114ns each (the (N+352)/1.2 formula; exps do NOT pipeline) = 142.6us
    busy.  v2 lost ~19.6us starting it (DMA-queue startup + cold PE) and
    ~31us to mid-stream stalls (S pairs queued behind up to 3.4us of
    statically-emitted filler in the in-order PE queue).
  - Bootstrap: 8 dummy warmup matmuls (no DMA deps) keep the PE busy through
    the HAM activity window so real work runs at 2.4GHz; first-needed weights
    (wqk0/wqk2/wv) ride the scalar-engine HW DMA queue (starts ~7us) instead
    of the slow gpsimd software queue (~9.5us); x pieces go tb-ordered on the
    sync queue.
  - Every slot emits its S pair FIRST, then fills the remaining PE budget
    (~1114ns/slot) with deadline-ordered half-units: A units (Q/K proj,
    8 matmuls) and B units (V proj) split into 4-matmul halves so no slot
    overflows and the next S pair never waits behind bulk filler.
  - D (P@[ones|V] accumulate) trails the exp stream by a VARIABLE lag: 12
    slots early (so the 16 B units + K-side A units fit in the early slots
    without stretching the exp chain), tapering to 3 by slot ~80 (so the
    tail stays short).  P tiles stage through a 16-deep bf16 SBUF ring.
  - fp8 was evaluated and rejected: attention output is a weighted mean
    whose magnitude shrinks exactly as fast as quantization noise, so e4m3
    anywhere (P, V, Q/K, or out-proj) costs 2-15% rel err vs the 2e-2 gate
    (numpy study).  Everything stays bf16 with f32 PSUM accumulation.
  - Kept from v2: exp-bias via per-kb [128,1] ACT bias operand; the
    [64 ones | 64 V] D stationary so PSUM rows 0:63 hold the replicated
    softmax denominator (reciprocal_approx_fast at PSUM base partition 0 +
    one fused multiply normalizes); out-proj PSUM->SBUF copies for the last
    q-block run on ACT (idle after the final exp).
"""

import numpy as np

import concourse.bass as bass  # noqa: F401
import concourse.mybir as mybir
import concourse.tile as tile
from concourse import bacc

F32 = mybir.dt.float32
BF16 = mybir.dt.bfloat16
EXP = mybir.ActivationFunctionType.Exp
NPBF16 = mybir.dt.np(BF16)

T = 2048
D = 1024
HPC = 4          # heads per core
HD = 64
NKB = T // 128   # 16 k/t blocks of 128
NQB = T // 512   # 4 q blocks of 512
NCH = D // 128   # 8 contraction chunks
SCALE = HD ** -0.5
BIAS_COEF = 0.1
RING = 20        # staged-P ring depth (must exceed max D lag+stagger + margin)

# per-slot PE budget model (warm ns)
SLOT_NS = 1114   # measured ACTIVATE cadence for [128,1024] exp
S_COST = 225
DHALF_COST = 230
AHALF_COST = 900
BHALF_COST = 540
EHALF_COST = 460

_NC_CACHE = {}


def _lag(i):
    """D drain lag: deep early (B units must fit before D consumes vsb),
    tapering through the deadline-free slots 30-57 so the pi=1 zone
    (A(*,3)/A(qb,1)/E demand) gets single drains, and the tail after the
    last exp stays ~3 slots."""
    if i <= 30:
        return 12
    return max(3, 12 - (i - 30) // 3)


def _ensure_patched_act_root():
    """Point walrus at an act_info.json with natural_log_exp_and_others
    listed first so Exp resolves to one table set (avoids ACT_TABLE_LOAD
    ping-pong)."""
    import json
    import os
    import tempfile

    if os.environ.get("BASS_ACT_ROOT_JSON_PATH"):
        return
    try:
        from neuronxcc.driver.Job import Job
        from neuronxcc.driver.jobs.support.FindActInfo import findActInfoFile

        src_json = findActInfoFile(Job.getPackageDir(), "gen3")
    except Exception:
        return
    try:
        d = os.path.dirname(src_json)
        out = tempfile.mkdtemp(prefix="act_root_")
        for f in os.listdir(d):
            os.symlink(os.path.join(d, f), os.path.join(out, f))
        with open(src_json) as fh:
            ai = json.load(fh)
        sets = ai.get("act_func_sets", [])
        nle = [s for s in sets if s["name"] == "natural_log_exp_and_others"]
        if not nle:
            return
        ai["act_func_sets"] = nle + [
            s for s in sets if s["name"] != "natural_log_exp_and_others"
        ]
        os.remove(os.path.join(out, "act_info.json"))
        with open(os.path.join(out, "act_info.json"), "w") as fh:
            json.dump(ai, fh)
        os.environ["BASS_ACT_ROOT_JSON_PATH"] = os.path.join(out, "act_info.json")
    except Exception:
        pass


def _build_nc(with_qkv_bias=False):
    _ensure_patched_act_root()
    nc = bacc.Bacc("TRN2", target_bir_lowering=False)

    # host pre-arranges everything into on-chip layouts (partition-major)
    xt_d = [
        nc.declare_dram_parameter(f"xt{tb}", [128, NCH, 512], BF16, isOutput=False)
        for tb in range(4)
    ]
    wqk_d = [
        nc.declare_dram_parameter(f"wqk{db}", [128, NCH, 128], BF16, isOutput=False)
        for db in range(4)
    ]
    wv_d = nc.declare_dram_parameter("wv", [128, NCH, 256], BF16, isOutput=False)
    wo_d = nc.declare_dram_parameter("wo", [128, 2, D], BF16, isOutput=False)
    bs_d = nc.declare_dram_parameter("bs", [128, NKB], F32, isOutput=False)
    bqk_d = nc.declare_dram_parameter("bqk", [1, 512], BF16, isOutput=False)
    bv_d = nc.declare_dram_parameter("bv", [1, 256], BF16, isOutput=False)
    ones_d = nc.declare_dram_parameter("ones", [1, 512], BF16, isOutput=False)
    y_d = nc.declare_dram_parameter("y", [T, D], BF16, isOutput=True)

    with tile.TileContext(nc) as tc:
        with (
            tc.tile_pool(name="const", bufs=1) as constp,
            tc.tile_pool(name="wts", bufs=1) as wts,
            tc.tile_pool(name="big", bufs=1) as bigp,
            tc.tile_pool(name="pt", bufs=RING) as ptp,
            tc.tile_pool(name="norm", bufs=2) as normp,

            tc.tile_pool(name="ysb", bufs=3) as ypool,
            tc.tile_pool(name="psG", bufs=2, space="PSUM") as psG,
            tc.tile_pool(name="psS", bufs=2, space="PSUM") as psS,
            tc.tile_pool(name="psO", bufs=1, space="PSUM") as psO,
        ):
            # ------------- DMAs ------------------------------------------
            # First-needed weights on the scalar-engine HW queue (fast
            # start, ACT is idle until the first exp anyway); the rest on
            # gpsimd; x tb-ordered in 2-chunk pieces on sync.
            # Bootstrap-critical transfers (wqk0/wqk2 + xt0) spread across
            # the three DMA queues: each queue runs only ~125-195 GB/s early
            # (2KB packets + limited outstanding-descriptor credit), so the
            # first A units' inputs must not share one queue.
            wqk_sb = [
                wts.tile([128, NCH, 128], BF16, tag=f"wqk{db}", name=f"wqk{db}")
                for db in range(4)
            ]
            wv_sb = wts.tile([128, NCH, 256], BF16, tag="wv")
            xT = [
                bigp.tile([128, NCH, 512], BF16, tag=f"xT{tb}", name=f"xT{tb}")
                for tb in range(4)
            ]
            bs_sb = constp.tile([128, NKB], F32, tag="bs")

            nc.scalar.dma_start(wqk_sb[0][:], wqk_d[0][:])
            nc.scalar.dma_start(wqk_sb[2][:], wqk_d[2][:])
            nc.gpsimd.dma_start(bs_sb[:], bs_d[:])
            # xt0: chunks 0-5 on sync, 6-7 on gpsimd (scalar still busy with
            # the wqk tiles the same A units need)
            for ch in range(0, 6, 2):
                nc.sync.dma_start(xT[0][:, ch:ch + 2, :], xt_d[0][:, ch:ch + 2, :])
            nc.gpsimd.dma_start(xT[0][:, 6:8, :], xt_d[0][:, 6:8, :])
            nc.scalar.dma_start(wv_sb[:], wv_d[:])
            for tb in range(1, 4):
                for ch in range(0, NCH, 2):
                    nc.sync.dma_start(
                        xT[tb][:, ch:ch + 2, :], xt_d[tb][:, ch:ch + 2, :]
                    )
            nc.gpsimd.dma_start(wqk_sb[1][:], wqk_d[1][:])
            nc.gpsimd.dma_start(wqk_sb[3][:], wqk_d[3][:])
            wo_sb = wts.tile([128, 2, D], BF16, tag="wo")
            nc.gpsimd.dma_start(wo_sb[:], wo_d[:])
            bqk_sb = wts.tile([1, 512], BF16, tag="bqk")
            nc.gpsimd.dma_start(bqk_sb[:], bqk_d[:])
            bv_sb = wts.tile([1, 256], BF16, tag="bv")
            nc.gpsimd.dma_start(bv_sb[:], bv_d[:])
            ones = constp.tile([1, 512], BF16, tag="ones")
            nc.gpsimd.dma_start(ones[:], ones_d[:])

            qkt = [
                bigp.tile([128, T], BF16, tag=f"qkt{db}", name=f"qkt{db}")
                for db in range(4)
            ]
            vsb = bigp.tile([128, NKB, HPC, 128], BF16, tag="vsb", name="vsb_v5")
            ot = [
                bigp.tile([128, T], BF16, tag=f"ot{pi}", name=f"ot{pi}")
                for pi in range(2)
            ]
            y_rows = y_d.rearrange("(n p) d -> n p d", p=128)

            # ------------- PE warmup -------------------------------------
            # ~3.4us of dummy matmuls (no DMA deps) during the DMA-queue
            # startup window: flips HAM to K=8/8 before the real A units.
            # The tiny dummy memset must be FIRST on the DVE queue (ahead of
            # the big vsb ones-memset) or the warmup waits ~7us behind it.
            dummy = constp.tile([128, 512], BF16, tag="dummy")
            nc.vector.memset(dummy[:], 0.0)
            nc.vector.memset(vsb[:], 1.0)
            for _ in range(10):
                wps = psG.tile([128, 512], F32, tag="gp", name="warm")
                nc.tensor.matmul(wps[:], dummy[:, 0:128], dummy[:], start=True,
                                 stop=True)

            # ------------- work units ------------------------------------
            def make_A(tb, db):
                st = {}

                def half0():
                    ps = psG.tile([128, 512], F32, tag="gp", name=f"qk{tb}_{db}")
                    st["ps"] = ps
                    for c in range(4):
                        nc.tensor.matmul(
                            ps[:], wqk_sb[db][:, c, :], xT[tb][:, c, :],
                            start=(c == 0), stop=False,
                        )

                def half1():
                    ps = st["ps"]
                    for c in range(4, NCH):
                        nc.tensor.matmul(
                            ps[:], wqk_sb[db][:, c, :], xT[tb][:, c, :],
                            start=False,
                            stop=(not with_qkv_bias and c == NCH - 1),
                        )
                    if with_qkv_bias:
                        nc.tensor.matmul(
                            ps[:],
                            bqk_sb[0:1, db * 128:(db + 1) * 128],
                            ones[0:1, :],
                            start=False, stop=True,
                        )
                    nc.vector.tensor_copy(
                        qkt[db][:, tb * 512:(tb + 1) * 512], ps[:]
                    )

                return [(half0, AHALF_COST), (half1, AHALF_COST)]

            def make_B(kb):
                tb, sub = divmod(kb, 4)
                st = {}

                def half0():
                    ps = psG.tile([128, 256], F32, tag="gp", name=f"v{kb}")
                    st["ps"] = ps
                    for c in range(4):
                        nc.tensor.matmul(
                            ps[:],
                            xT[tb][:, c, sub * 128:(sub + 1) * 128],
                            wv_sb[:, c, :],
                            start=(c == 0), stop=False,
                        )

                def half1():
                    ps = st["ps"]
                    for c in range(4, NCH):
                        nc.tensor.matmul(
                            ps[:],
                            xT[tb][:, c, sub * 128:(sub + 1) * 128],
                            wv_sb[:, c, :],
                            start=False,
                            stop=(not with_qkv_bias and c == NCH - 1),
                        )
                    if with_qkv_bias:
                        nc.tensor.matmul(
                            ps[:], ones[0:1, 0:128], bv_sb[:],
                            start=False, stop=True,
                        )
                    nc.vector.tensor_copy(
                        vsb[:, kb, :, 64:128],
                        ps[:].rearrange("p (h c) -> p h c", h=HPC),
                    )

                return [(half0, BHALF_COST), (half1, BHALF_COST)]

            def emit_S_exp(qb, pi, kb):
                qdb, kdb = pi, 2 + pi
                s01 = psS.tile([128, 1024], F32, tag="s01", name=f"s{qb}_{pi}_{kb}")
                nc.tensor.matmul(
                    s01[:, 0:512],
                    qkt[kdb][0:64, kb * 128:(kb + 1) * 128],
                    qkt[qdb][0:64, qb * 512:(qb + 1) * 512],
                )
                nc.tensor.matmul(
                    s01[:, 512:1024],
                    qkt[kdb][64:128, kb * 128:(kb + 1) * 128],
                    qkt[qdb][64:128, qb * 512:(qb + 1) * 512],
                )
                p01 = ptp.tile([128, 1024], BF16, tag="p01", name=f"p{qb}_{pi}_{kb}")
                nc.scalar.activation(p01[:], s01[:], EXP, bias=bs_sb[:, kb:kb + 1])
                return p01

            def emit_D_half(qb, pi, kb, parity, p01, os):
                # stationary is [128 kpos, 64 ones-columns + 64 vdims]: rows
                # 0:63 of the accumulator get the softmax denominator already
                # replicated across 64 partitions (same N-cycle cost).
                nc.tensor.matmul(
                    os[:], vsb[:, kb, 2 * pi + parity, :],
                    p01[:, 512 * parity:512 * parity + 512],
                    start=(kb == 0), stop=(kb == NKB - 1),
                )

            def emit_stage_parity(qb, pi, parity, os):
                # Normalize one head straight out of PSUM: reciprocal of the
                # replicated denominator (rows 0:63 -> PSUM base partition 0,
                # required by custom-DVE ucode ops on HW), then one fused
                # multiply reading O^T from PSUM rows 64:128.  Two DVE ops
                # per head; the next group's same-parity accumulator is
                # WAR-released when the multiply completes.
                cols = slice(qb * 512, (qb + 1) * 512)
                recd = normp.tile(
                    [64, 512], F32, tag="recd", name=f"recd{qb}_{2*pi+parity}"
                )
                nc.vector.reciprocal_approx_fast(out=recd[:], in_=os[0:64, :])
                nc.vector.tensor_mul(
                    ot[pi][64 * parity:64 * parity + 64, cols],
                    os[64:128, :],
                    recd[:],
                )

            ysb_tiles = {}

            def emit_E_half(tb, nb, act_copy=False, dma_half=False):
                yps = psG.tile([128, 512], F32, tag="gp", name=f"yps{tb}_{nb}")
                nc.tensor.matmul(
                    yps[:], ot[0][:, tb * 128:(tb + 1) * 128],
                    wo_sb[:, 0, nb * 512:(nb + 1) * 512],
                    start=True, stop=False,
                )
                nc.tensor.matmul(
                    yps[:], ot[1][:, tb * 128:(tb + 1) * 128],
                    wo_sb[:, 1, nb * 512:(nb + 1) * 512],
                    start=False, stop=True,
                )
                if nb == 0:
                    ysb_tiles[tb] = ypool.tile([128, D], BF16, tag="ysb",
                                               name=f"ysb{tb}")
                ysb = ysb_tiles[tb]
                if act_copy:
                    nc.scalar.activation(
                        ysb[:, nb * 512:(nb + 1) * 512], yps[:],
                        mybir.ActivationFunctionType.Copy,
                    )
                else:
                    nc.vector.tensor_copy(ysb[:, nb * 512:(nb + 1) * 512], yps[:])
                if dma_half:
                    nc.sync.dma_start(
                        y_rows[tb][:, nb * 512:(nb + 1) * 512],
                        ysb[:, nb * 512:(nb + 1) * 512],
                    )
                elif nb == 1:
                    nc.sync.dma_start(y_rows[tb], ysb[:])

            # ------------- filler unit queue (deadline-ordered) ----------
            # deadline = slot by which the LAST half must have executed;
            # avail = earliest slot the unit's xT DMA has landed (estimate).
            # qb-major group order: all pi=0 groups first, so the pi=1
            # projection units (A(*,3), A(qb,1)) defer past slot 64 and the
            # early window only has to fit B + the pi=0 A units.
            AVAIL_TB = [0, 0, 2, 5]
            units = []  # (deadline, avail, [(closure, cost), ...])
            for kb in range(NKB):
                units.append((kb + 10, AVAIL_TB[kb // 4], make_B(kb)))
            for tb in range(1, 4):
                units.append((4 * tb - 2, AVAIL_TB[tb], make_A(tb, 2)))
            for qb in range(1, 4):
                units.append((16 * qb - 4, AVAIL_TB[qb], make_A(qb, 0)))
            for tb in range(4):
                units.append((60 + 4 * tb, AVAIL_TB[tb], make_A(tb, 3)))
            for qb in range(4):
                units.append((16 * (4 + qb) - 4, AVAIL_TB[qb], make_A(qb, 1)))
            units.sort(key=lambda u: u[0])
            uq = [
                {"deadline": d, "avail": a, "pieces": list(p)}
                for d, a, p in units
            ]

            epilogue = []  # E halves, appended as groups drain

            def pop_filler(i, forced_only):
                """Next filler piece in deadline order.  A partially-emitted
                unit is always at the queue head (avail is static), so its
                remaining pieces pop before any other unit opens a psG tile.
                Untouched units whose xT DMA hasn't landed are skipped."""
                for u in uq:
                    if not u["pieces"]:
                        continue
                    if u["avail"] > i:
                        continue
                    if forced_only and u["deadline"] > i + 1:
                        return None
                    return u["pieces"].pop(0)
                if forced_only:
                    return None
                return epilogue.pop(0) if epilogue else None

            # ------------- schedule --------------------------------------
            groups = [(0, 0), (1, 0), (2, 0), (3, 0), (0, 1), (1, 1),
                      (2, 1), (3, 1)]
            slots = [(qb, pi, kb) for (qb, pi) in groups for kb in range(NKB)]

            ring = {}
            osAB = {}
            tail_reserve = []  # E halves held back to cover the tail's
            #                    stage-out DVE window (keeps HAM warm)

            def open_group(gidx):
                qb, pi = groups[gidx]
                osAB[(qb, pi)] = (
                    psO.tile([128, 512], F32, tag="osA", name=f"osA{qb}_{pi}"),
                    psO.tile([128, 512], F32, tag="osB", name=f"osB{qb}_{pi}"),
                )

            def drain_half(h):
                j, parity = divmod(h, 2)
                jqb, jpi, jkb = slots[j]
                os = osAB[(jqb, jpi)][parity]
                emit_D_half(jqb, jpi, jkb, parity, ring[j], os)
                if parity == 1:
                    ring.pop(j)
                if jkb != NKB - 1:
                    return
                emit_stage_parity(jqb, jpi, parity, os)
                if parity != 1:
                    return
                del osAB[(jqb, jpi)]
                gidx = groups.index((jqb, jpi))
                if gidx + 1 < len(groups):
                    open_group(gidx + 1)
                if jpi == 1:
                    last = jqb == NQB - 1
                    halves = []
                    for sub in range(4):
                        for nb in range(2):
                            if last:
                                # tail: alternate ACT/DVE so the serialized
                                # copy chain doesn't gate the psG ring, and
                                # DMA per half so output transfers overlap
                                ac = (2 * sub + nb) % 2 == 0
                                halves.append((
                                    lambda t=4 * jqb + sub, nb=nb, a=ac:
                                    emit_E_half(t, nb, act_copy=a,
                                                dma_half=True),
                                    EHALF_COST,
                                ))
                            else:
                                ac = jqb == 2 and 2 * sub + nb >= 5
                                halves.append((
                                    lambda t=4 * jqb + sub, nb=nb, a=ac:
                                    emit_E_half(t, nb, act_copy=a),
                                    EHALF_COST,
                                ))
                    if jqb == 2:
                        epilogue.extend(halves[:5])
                        tail_reserve.extend(halves[5:])
                    else:
                        epilogue.extend(halves)

            # bootstrap: A(0,0) + A(0,2) interleaved chunk-wise (consume x
            # pieces as they land), then open the first accumulator group.
            psA0 = psG.tile([128, 512], F32, tag="gp", name="qk0_0")
            psA2 = psG.tile([128, 512], F32, tag="gp", name="qk0_2")
            for c in range(NCH):
                last = not with_qkv_bias and c == NCH - 1
                nc.tensor.matmul(psA0[:], wqk_sb[0][:, c, :], xT[0][:, c, :],
                                 start=(c == 0), stop=last)
                nc.tensor.matmul(psA2[:], wqk_sb[2][:, c, :], xT[0][:, c, :],
                                 start=(c == 0), stop=last)
            if with_qkv_bias:
                nc.tensor.matmul(psA0[:], bqk_sb[0:1, 0:128], ones[0:1, :],
                                 start=False, stop=True)
                nc.tensor.matmul(psA2[:], bqk_sb[0:1, 256:384], ones[0:1, :],
                                 start=False, stop=True)
            nc.vector.tensor_copy(qkt[0][:, 0:512], psA0[:])
            nc.vector.tensor_copy(qkt[2][:, 0:512], psA2[:])
            open_group(0)

            n = len(slots)

            def half_due(h, i):
                j, parity = divmod(h, 2)
                kb = slots[j][2]
                # kb==0 halves stagger +2/+3 slots past the lag: parity A
                # waits the previous group's recipA+mulA, parity B its
                # recipB+mulB (each ~1.4us of serialized DVE).
                extra = (2 + parity) if kb == 0 else 0
                return j + extra <= i - _lag(i)

            drain_h = 0
            debt = 0
            for i, (qb, pi, kb) in enumerate(slots):
                budget = SLOT_NS + debt
                ring[i] = emit_S_exp(qb, pi, kb)
                budget -= S_COST
                # overdue filler runs regardless of budget
                while True:
                    f = pop_filler(i, forced_only=True)
                    if f is None:
                        break
                    f[0]()
                    budget -= f[1]
                # due D half-drains
                while drain_h < 2 * n and half_due(drain_h, i):
                    drain_half(drain_h)
                    drain_h += 1
                    budget -= DHALF_COST
                # optional fill
                while budget > 0:
                    f = pop_filler(i, forced_only=False)
                    if f is None:
                        break
                    f[0]()
                    budget -= f[1]
                debt = min(0, budget)

            # tail: drain the remaining halves (final stage chain lands on
            # DVE), cover its window with the reserved E halves, then the
            # last q-block's out-proj.
            while drain_h < 2 * n:
                drain_half(drain_h)
                drain_h += 1
            for f in tail_reserve:
                f[0]()
            while epilogue:
                epilogue.pop(0)[0]()

    nc.compile()
    return nc


def _get_nc(with_qkv_bias=False):
    key = ("nc", bool(with_qkv_bias))
    if key not in _NC_CACHE:
        _NC_CACHE[key] = _build_nc(bool(with_qkv_bias))
    return _NC_CACHE[key]


def _arrange_pmajor(w, cols):
    """[D, cols] -> [128, D//128, cols] partition-major bf16."""
    return np.ascontiguousarray(
        w.reshape(NCH, 128, cols).transpose(1, 0, 2)
    ).astype(NPBF16)


def _make_in_maps(x, boundary_score, W_qkv, b_qkv, W_out):
    x = np.asarray(x, np.float32)
    boundary_score = np.asarray(boundary_score, np.float32)
    W_qkv = np.asarray(W_qkv, np.float32)
    b_qkv = np.asarray(b_qkv, np.float32)
    W_out = np.asarray(W_out, np.float32)

    Wq, Wk, Wv = W_qkv[:, :D], W_qkv[:, D:2 * D], W_qkv[:, 2 * D:]
    bq, bk, bv = b_qkv[:D], b_qkv[D:2 * D], b_qkv[2 * D:]
    ones = np.ones((1, 512), NPBF16)

    # x^T arranged [128, NCH, T] bf16 then split into 4 tb blocks
    xts = []
    for b in range(x.shape[0]):
        xt = np.ascontiguousarray(
            x[b].T.reshape(NCH, 128, T).transpose(1, 0, 2)
        ).astype(NPBF16)
        xts.append([np.ascontiguousarray(xt[:, :, tb * 512:(tb + 1) * 512])
                    for tb in range(4)])

    in_maps = []
    for c in range(8):
        b, g = divmod(c, 4)
        lo, hi = 256 * g, 256 * (g + 1)
        wqk = _arrange_pmajor(
            np.concatenate([Wq[:, lo:hi] * SCALE, Wk[:, lo:hi]], axis=1), 512
        )
        wqks = {
            f"wqk{db}": np.ascontiguousarray(wqk[:, :, db * 128:(db + 1) * 128])
            for db in range(4)
        }
        bqk = np.concatenate([bq[lo:hi] * SCALE, bk[lo:hi]])[None].astype(NPBF16)
        wv = _arrange_pmajor(Wv[:, lo:hi], 256)
        bvv = np.ascontiguousarray(bv[lo:hi][None]).astype(NPBF16)
        wo = np.ascontiguousarray(
            W_out[lo:hi, :].reshape(2, 128, D).transpose(1, 0, 2)
        ).astype(NPBF16)
        bs = np.ascontiguousarray(
            (boundary_score[b] * BIAS_COEF).reshape(NKB, 128).T
        )
        m = dict(
            bqk=np.ascontiguousarray(bqk), wv=wv, bv=bvv, wo=wo,
            bs=bs, ones=ones, **wqks,
        )
        for tb in range(4):
            m[f"xt{tb}"] = xts[b][tb]
        in_maps.append(m)
    return in_maps


def kernel(x, boundary_score, W_qkv, b_qkv, W_out, b_out):
    from concourse.bass_utils import run_bass_kernel_spmd

    x = np.asarray(x, np.float32)
    B = x.shape[0]
    in_maps = _make_in_maps(x, boundary_score, W_qkv, b_qkv, W_out)
    nc = _get_nc(with_qkv_bias=bool(np.any(np.asarray(b_qkv))))
    res = run_bass_kernel_spmd(nc, in_maps, list(range(8))).results
    out = np.zeros((B, T, D), np.float32)
    for c in range(8):
        out[c // 4] += res[c]["y"].astype(np.float32)
    out += np.asarray(b_out, np.float32)
    return out
